# revision 4
# baseline (speedup 1.0000x reference)
"""BatchAllTripletLoss Trainium2 kernel, v3 (transposed big-FD epilogue).

Problem (hardcoded): x (64, 256, 256) f32, y (64, 256) int64 with
y[p, i] = i // 8 (32 classes x 8 members). Output: per-part batch-all
triplet loss, shape (64,) f32.

Key ideas vs baseline:
  - D is symmetric, so the dmat tiles ([anchors 128h.., all cols]) are
    also the transposed chunks ([cols 128c.., all anchors]).
  - Build TAU[l, (t,i)] = tau(i,t) - 16 (bf16, identical rows) via a
    DRAM-bounce broadcast DMA, then ONE tensor_tensor per chunk:
      E[l, (t,i)] = (D[l,i]-16) - (tau(i,t)-16) = D - tau
    at DVE 2x bf16 (0.59 ns/free-elem) instead of 16 per-slot ops.
  - Fused big-FD reductions over E (per partition l = per column):
      relu-sum: ACT Relu(-E) + accum   count: ACT Sign(-E) + accum or
      DVE tensor_scalar is_lt-0 + accum.
  - -16 range shift halves bf16 rounding noise of the inputs to E.
  - Pollution (+L on same-class cols, L=2^20) makes polluted D exactly
    1024.0 (1008.0 after shift) in bf16 -> pos columns contribute
    exactly 0 to relu-sum and count.
  - Host-side finale from raw per-(part,pass) sums.
"""

import numpy as np
from contextlib import ExitStack

import concourse.bass as bass
import concourse.bacc as bacc_mod
import concourse.mybir as mybir
import concourse.tile as tile

F32 = mybir.dt.float32
BF16 = mybir.dt.bfloat16
ALU = mybir.AluOpType
ACTF = mybir.ActivationFunctionType

P_TOT, N, C = 64, 256, 256
K, NCLS = 8, 32
MARGIN = 0.2
SHIFT = 16.0
NCORES = 8
PPC = P_TOT // NCORES
HALVES = 2
LBIG = float(1 << 20)  # sqrt -> 1024.0 exact in bf16 (1008 after shift)
EPS = 4.0
ACC_W = 4  # cols: 0 = relu-sum c0c1 (ACT), 1 = cnt chunk0 (DVE), 2 = sign chunk1 (ACT), 3 = spare


def build_kernel(do_compile=True):
    nc = bacc_mod.Bacc()
    x_in = nc.declare_dram_parameter("x", [PPC * N, C], F32, isOutput=False)
    sn_out = nc.declare_dram_parameter("sn", [1, ACC_W * PPC], F32, isOutput=True)
    # DRAM scratch for the tau-row broadcast (double buffered across parts)
    scr = [nc.dram_tensor(f"tauscr{i}", [1, K * N], BF16, kind="Internal") for i in range(4)]

    with tile.TileContext(nc) as tc, ExitStack() as ctx:
        consts = ctx.enter_context(tc.tile_pool(name="consts", bufs=1))
        xpool = ctx.enter_context(tc.tile_pool(name="xpool", bufs=2))
        xtpool = ctx.enter_context(tc.tile_pool(name="xtpool", bufs=2))
        dpool = ctx.enter_context(tc.tile_pool(name="dpool", bufs=3))
        epool = ctx.enter_context(tc.tile_pool(name="epool", bufs=3))
        taupool = ctx.enter_context(tc.tile_pool(name="taupool", bufs=3))
        small = ctx.enter_context(tc.tile_pool(name="small", bufs=3))
        trash = ctx.enter_context(tc.tile_pool(name="trash", bufs=4))
        accp = ctx.enter_context(tc.tile_pool(name="accp", bufs=3))
        psum = ctx.enter_context(tc.tile_pool(name="psum", bufs=2, space="PSUM"))
        psmall = ctx.enter_context(tc.tile_pool(name="psmall", bufs=2, space="PSUM"))
        pfin = ctx.enter_context(tc.tile_pool(name="pfin", bufs=1, space="PSUM"))
        pcnt = ctx.enter_context(tc.tile_pool(name="pcnt", bufs=1, space="PSUM"))

        # ---- constants ----
        ct_one = consts.tile([NCLS, N], BF16, tag="ct1")
        nc.vector.memset(ct_one[:], 1.0)
        nc.gpsimd.affine_select(
            ct_one[:], ct_one[:], pattern=[[1, NCLS], [0, K]],
            compare_op=ALU.is_equal, fill=0.0, base=0, channel_multiplier=-1,
        )
        ct_a = consts.tile([NCLS, N], BF16, tag="cta")  # -L/2 * B
        nc.vector.memset(ct_a[:], -LBIG / 2)
        nc.gpsimd.affine_select(
            ct_a[:], ct_a[:], pattern=[[1, NCLS], [0, K]],
            compare_op=ALU.is_equal, fill=0.0, base=0, channel_multiplier=-1,
        )
        ct_b = consts.tile([NCLS, N], BF16, tag="ctb")  # +L * B
        nc.vector.memset(ct_b[:], LBIG)
        nc.gpsimd.affine_select(
            ct_b[:], ct_b[:], pattern=[[1, NCLS], [0, K]],
            compare_op=ALU.is_equal, fill=0.0, base=0, channel_multiplier=-1,
        )
        ct_half = consts.tile([NCLS, N], BF16, tag="cth")  # B - 0.5
        nc.vector.memset(ct_half[:], 0.5)
        nc.gpsimd.affine_select(
            ct_half[:], ct_half[:], pattern=[[1, NCLS], [0, K]],
            compare_op=ALU.is_equal, fill=-0.5, base=0, channel_multiplier=-1,
        )
        ident = consts.tile([128, 128], BF16, tag="ident")
        nc.vector.memset(ident[:], 1.0)
        nc.gpsimd.affine_select(
            ident[:], ident[:], pattern=[[1, 128]],
            compare_op=ALU.is_equal, fill=0.0, base=0, channel_multiplier=-1,
        )
        identf = consts.tile([128, 128], F32, tag="identf")
        nc.vector.memset(identf[:], 1.0)
        nc.gpsimd.affine_select(
            identf[:], identf[:], pattern=[[1, 128]],
            compare_op=ALU.is_equal, fill=0.0, base=0, channel_multiplier=-1,
        )
        neghalf = consts.tile([1, 128], BF16, tag="neghalf")
        nc.vector.memset(neghalf[:], -0.5)
        ieps = consts.tile([128, 128], BF16, tag="ieps")
        nc.vector.memset(ieps[:], -EPS / 2)
        nc.gpsimd.affine_select(
            ieps[:], ieps[:], pattern=[[1, 128]],
            compare_op=ALU.is_equal, fill=0.0, base=0, channel_multiplier=-1,
        )
        ishift = []
        for h in range(HALVES):
            t_ish = consts.tile([128, N], BF16, tag=f"ish{h}", name=f"ish{h}")
            nc.vector.memset(t_ish[:], 1.0)
            nc.gpsimd.affine_select(
                t_ish[:], t_ish[:], pattern=[[1, N]],
                compare_op=ALU.is_equal, fill=0.0, base=-128 * h,
                channel_multiplier=-1,
            )
            ishift.append(t_ish)
        ones_col = consts.tile([128, 1], F32, tag="ones_col")
        nc.vector.memset(ones_col[:], 1.0)
        # combo lhsT [33, N]: rows 0-31 = -L/2 * class indicator, row 32 = -0.5
        lhsT_cat = consts.tile([33, N], BF16, tag="lhsT_cat")
        nc.vector.memset(lhsT_cat[:], -LBIG / 2)
        nc.gpsimd.affine_select(
            lhsT_cat[0:NCLS, :], lhsT_cat[0:NCLS, :], pattern=[[1, NCLS], [0, K]],
            compare_op=ALU.is_equal, fill=0.0, base=0, channel_multiplier=-1,
        )
        nc.vector.memset(lhsT_cat[32:33, :], -0.5)
        # combo rhs pair [33, N]: rows 0-31 = ct_one, row 32 = sqrow (per part)
        rhs_cat = []
        for i in range(2):
            rt = consts.tile([33, N], BF16, tag=f"rhs_cat{i}", name=f"rhs_cat{i}")
            nc.vector.memset(rt[:], 1.0)
            nc.gpsimd.affine_select(
                rt[0:NCLS, :], rt[0:NCLS, :], pattern=[[1, NCLS], [0, K]],
                compare_op=ALU.is_equal, fill=0.0, base=0, channel_multiplier=-1,
            )
            rhs_cat.append(rt)
        onesb = consts.tile([128, 1], BF16, tag="onesb")
        nc.vector.memset(onesb[:], 1.0)

        fin_ps = pfin.tile([1, ACC_W * PPC], F32, tag="fin_ps")

        for p in range(PPC):
            # ---- load rows f32: one DMA for both halves ----
            xf2 = xpool.tile([128, 2 * C], F32, tag="xf", name="xf", bufs=2)
            nc.sync.dma_start(
                xf2[:].rearrange("i (h c) -> i h c", h=2, c=C),
                x_in[p * N: (p + 1) * N, :].rearrange("(h i) c -> i h c", h=2, i=128),
            )
            xf = [xf2[:, 0:C], xf2[:, C: 2 * C]]
            # cast rows bf16 on ACT, transpose bf16 via PE (1 cyc/row)
            xb2 = xpool.tile([128, 2 * C], BF16, tag="xb2", name="xb2")
            nc.scalar.activation(xb2[:], xf2[:], ACTF.Copy, bias=0.0, scale=1.0)
            xb = [xb2[:, 0:C], xb2[:, C: 2 * C]]
            xtps = psum.tile([128, 2 * N], BF16, tag="xtps", name="xtps", bufs=2)
            for cchunk in range(2):
                for h in range(HALVES):
                    nc.tensor.transpose(
                        xtps[:, 256 * cchunk + 128 * h: 256 * cchunk + 128 * (h + 1)],
                        xb[h][:, 128 * cchunk: 128 * (cchunk + 1)],
                        ident[:],
                    )
            xtb_all = xtpool.tile([128, 2 * N], BF16, tag="xtb", name="xtb")
            nc.vector.tensor_copy(xtb_all[:], xtps[:])
            xtb = [xtb_all[:, 0:N], xtb_all[:, N: 2 * N]]

            # ---- squared norms: DVE stt square-accum on f32 rows ----
            sqcol = []
            sqcol_b = []
            for h in range(HALVES):
                sc = small.tile([128, 1], F32, tag="sqcol")
                st = trash.tile([128, C], BF16, tag="trash_sq")
                nc.vector.scalar_tensor_tensor(
                    st[:], xf[h][:], 0.0, xf[h][:],
                    op0=ALU.bypass, op1=ALU.mult, accum_out=sc[:],
                )
                sqcol.append(sc)
                scb = small.tile([128, 1], BF16, tag="sqcolb")
                nc.vector.tensor_copy(scb[:], sc[:])
                sqcol_b.append(scb)
            strip_ps = psmall.tile([33, N], BF16, tag="strip_ps", name="strip_ps")
            for h in range(HALVES):
                nc.tensor.transpose(
                    strip_ps[32:33, 128 * h: 128 * (h + 1)], sqcol_b[h][:], ident[:]
                )
            rc = rhs_cat[p % 2]
            nc.vector.tensor_copy(rc[32:33, :], strip_ps[32:33, :])

            # ---- per half: gram psum, D, pm ----
            dmat_s = []
            pmb = []
            for h in range(HALVES):
                ps = psum.tile([128, N], F32, tag="ps")
                nc.tensor.matmul(
                    ps[:], xtb[0][:, 128 * h: 128 * (h + 1)], xtb[0][:],
                    start=True, stop=False,
                )
                nc.tensor.matmul(
                    ps[:], xtb[1][:, 128 * h: 128 * (h + 1)], xtb[1][:],
                    start=False, stop=False,
                )
                nc.tensor.matmul(
                    ps[:], lhsT_cat[:, 128 * h: 128 * (h + 1)], rc[:],
                    start=False, stop=True,
                )
                dmf = dpool.tile([128, N], F32, tag="dmf")
                nc.scalar.activation(
                    dmf[:], ps[:], ACTF.Sqrt, bias=sqcol[h][:], scale=-2.0,
                )
                dms = dpool.tile([128, N], BF16, tag="dms")
                nc.vector.tensor_scalar(dms[:], dmf[:], SHIFT, None, op0=ALU.subtract)
                dmat_s.append(dms)

                # pos extraction (flip trick)
                nc.tensor.matmul(
                    ps[:], ct_b[:, 128 * h: 128 * (h + 1)], ct_half[:],
                    start=False, stop=False, skip_group_check=True,
                )
                nc.tensor.matmul(
                    ps[:], ieps[:], ishift[h][:],
                    start=False, stop=True, skip_group_check=True,
                )
                spos = small.tile([128, K], F32, tag="spos")
                nc.vector.tensor_reduce(
                    spos[:],
                    ps[:].rearrange("p (h t) -> p t h", h=NCLS, t=K),
                    axis=mybir.AxisListType.X, op=ALU.max,
                )
                pp = small.tile([128, K], F32, tag="pp")
                nc.scalar.activation(pp[:], spos[:], ACTF.Sqrt, bias=sqcol[h][:], scale=-2.0)
                # pm_shift bf16 = pp + (margin - 16)
                pmbh = small.tile([128, K], BF16, tag="pmb")
                nc.vector.tensor_scalar(pmbh[:], pp[:], MARGIN - SHIFT, None, op0=ALU.add)
                pmb.append(pmbh)

            # ---- tau row [8, 256] and TAU [128, 2048] broadcast ----
            for h in range(HALVES):
                nc.tensor.transpose(
                    strip_ps[0:K, 128 * h: 128 * (h + 1)], pmb[h][:], ident[:]
                )
            pmrow = small.tile([K, N], BF16, tag="pmrow")
            nc.vector.tensor_copy(pmrow[:], strip_ps[0:K, :])
            s = scr[p % 4]
            nc.sync.dma_start(
                s[0:1, :].rearrange("o (t l) -> (o t) l", t=K, l=N),
                pmrow[:],
            )
            tau = taupool.tile([128, K * N], BF16, tag="tau", name="tau")
            nc.sync.dma_start(tau[:], s[0:1, :].broadcast_to([128, K * N]))

            # ---- E = D - tau per chunk into one [128, 4096] tile ----
            eall = epool.tile([128, 2 * K * N], BF16, tag="eall", name="eall")
            for c in range(2):
                nc.vector.tensor_tensor(
                    eall[:, 2048 * c: 2048 * (c + 1)].rearrange("p (t i) -> p t i", t=K, i=N),
                    dmat_s[c][:].rearrange("p (o i) -> p o i", o=1, i=N).broadcast_to([128, K, N]),
                    tau[:].rearrange("p (t i) -> p t i", t=K, i=N),
                    op=ALU.subtract,
                )

            # ---- fused reductions ----
            acc = accp.tile([128, ACC_W], F32, tag="acc", name="acc")
            # relu-sum per chunk (pipelines with the tt ops)
            for c in range(2):
                tr1 = trash.tile([128, K * N], BF16, tag="tr1")
                nc.scalar.activation(
                    tr1[:], eall[:, 2048 * c: 2048 * (c + 1)], ACTF.Relu,
                    bias=0.0, scale=-1.0, accum_out=acc[:, c: c + 1],
                )
            # counts: indicator tiles on DVE (plain 4x), column-summed by PE
            cnt_ps = pcnt.tile([128, 1], F32, tag="cnt_ps", name="cnt_ps")
            for c in range(2):
                ind = trash.tile([128, K * N], BF16, tag=f"ind{c}", name=f"ind{c}")
                nc.vector.tensor_scalar(
                    ind[:], eall[:, 2048 * c: 2048 * (c + 1)], 0.0, None,
                    op0=ALU.is_lt,
                )
                for k in range(16):
                    nc.tensor.matmul(
                        cnt_ps[:], ind[:, 128 * k: 128 * (k + 1)], onesb[:],
                        start=(c == 0 and k == 0), stop=(c == 1 and k == 15),
                    )
            nc.vector.tensor_copy(acc[:, 2:3], cnt_ps[:])
            nc.vector.memset(acc[:, 3:4], 0.0)

            # ---- cross-partition sums ----
            nc.tensor.matmul(
                fin_ps[0:1, ACC_W * p: ACC_W * (p + 1)], ones_col[:], acc[:],
                start=True, stop=True,
            )

        fin = small.tile([1, ACC_W * PPC], F32, tag="fin")
        nc.vector.tensor_copy(fin[:], fin_ps[:])
        nc.sync.dma_start(sn_out[:], fin[:])

    if do_compile:
        nc.compile()
    return nc


_NC_CACHE = None


def _get_nc():
    global _NC_CACHE
    if _NC_CACHE is None:
        _NC_CACHE = build_kernel()
    return _NC_CACHE


def kernel(x: np.ndarray, y: np.ndarray) -> np.ndarray:
    from concourse.bass_utils import run_bass_kernel_spmd

    x = np.asarray(x)
    y = np.asarray(y)
    assert x.shape == (P_TOT, N, C) and y.shape == (P_TOT, N)
    expect = np.repeat(np.arange(NCLS, dtype=np.int64), K)
    assert np.array_equal(y, np.broadcast_to(expect, (P_TOT, N))), (
        "kernel requires y[p, i] == i // 8"
    )
    nc = _get_nc()
    xs = np.ascontiguousarray(x.reshape(NCORES, PPC * N, C).astype(np.float32))
    in_maps = [{"x": xs[i]} for i in range(NCORES)]
    res = run_bass_kernel_spmd(nc, in_maps, list(range(NCORES)))
    out = np.empty((P_TOT,), np.float32)
    for i in range(NCORES):
        fin = res.results[i]["sn"].reshape(PPC, ACC_W).astype(np.float64)
        relu_sum = fin[:, 0] + fin[:, 1]
        cnt = fin[:, 2]
        loss = np.where(cnt <= 0, 0.0, relu_sum / np.maximum(cnt, 1.0))
        out[i * PPC: (i + 1) * PPC] = loss.astype(np.float32)
    return out
